# revision 1
# baseline (speedup 1.0000x reference)
"""AnalyticGaussianVelocity (soft-kNN flow velocity) on 8 trn2 NeuronCores.

Math (reference):
    a = t, b = 1-t
    logit[b,n] = -1/(2 b^2) * ||x_b - a * d_n||^2
    prob = softmax(logit, axis=n) * (1 + a/b)
    v = (-1/b) x + prob @ dataset

Dropping per-row constants, softmax(logit) == softmax(u * P) with
    u = a/b^2  (>0),  P[b,n] = x_b . d_n - (a/2) ||d_n||^2

Kernel strategy (dataset sharded over N across 8 cores, flash-style
online softmax per core, AllReduce merge):
  MM1: P = x^T . dataT as a 3-pass hi/lo bf16 split ("bsplit" default,
       1 cyc/row and interleave-safe; "fp32" = plain fp32 at 4 cyc/row;
       "split" = 3-pass split-float32r - fastest per-op but f32r
       accumulation groups get corrupted when fp32 transpose-mode PE ops
       interleave with them, do not enable without revalidating) +
       a K=6 matmul folding in the -(a/2)||d||^2 term from 3-way bf16
       splits of w and of the dataset norms (norms via fp32 ones-matmul
       on squared transposed chunks).
  softmax: DVE row-max -> ACT exp(scale=u, bias=-u*m) with free row-sum
       (accum_out), prob emitted in bf16.
  MM2: acc_new = diag(alpha) @ acc (f32r rescale matmul) + probT @ dataset
       (bf16); probT via PE transposes (xbar DMA transpose races when
       pipelined - keep USE_XBAR=False).
  merge: AllReduce-max of m, rescale by exp(u(m_loc-m_glob)),
         AllReduce-add of [acc | l], then v = dcoef*acc/l + vcoef*x.
"""

import sys

sys.path.insert(0, "/opt/trn_rl_repo")

import numpy as np

import concourse.bass as bass
import concourse.mybir as mybir
import concourse.tile as tile
from concourse import bacc
from concourse.bass_utils import run_bass_kernel_spmd
from concourse.masks import make_identity

B, D = 1024, 512
NCORES = 8
NTILE = 512  # dataset rows per n-tile
NBT = B // 128  # 8 b-tiles

F32 = mybir.dt.float32
F32R = mybir.dt.float32r
BF16 = mybir.dt.bfloat16

AF = mybir.ActivationFunctionType
OP = mybir.AluOpType
AX = mybir.AxisListType

DEBUG = False
USE_XBAR = False
LINEARIZE = False
MM1_MODE = "bsplit"  # "fp32" | "split" | "bsplit"
SIM_1CORE = False  # build single-core, no collectives (for TimelineSim)
SIM_SKIP = set()  # sim-only op omission for time attribution
BUFS_NAT = 2
BUFS_DT = 2
BUFS_SF = 5
BUFS_DN = 2
BUFS_TINY = 4
ACC_COPY_DVE = False
BUFS_PSL = 3
BUFS_PSA = 2
BUFS_PST = 2
BUFS_SQ = 2


def build(n_tiles):
    n_sh = n_tiles * NTILE
    split = MM1_MODE in ("split", "bsplit")
    SDT = BF16 if MM1_MODE == "bsplit" else F32R  # split operand dtype
    ndev = 1 if SIM_1CORE else NCORES
    nc = bacc.Bacc("TRN2", target_bir_lowering=False, debug=False, num_devices=ndev)

    ds = nc.declare_dram_parameter("dataset", [n_sh, D], F32, isOutput=False)
    xt = nc.declare_dram_parameter("x_t", [B, D], F32, isOutput=False)
    # per-b coefficient vectors, column layout [128, 8]: col i holds b = i*128+p
    ucol_p = nc.declare_dram_parameter("ucol", [128, NBT], F32, isOutput=False)
    nucol_p = nc.declare_dram_parameter("nucol", [128, NBT], F32, isOutput=False)
    dcol_p = nc.declare_dram_parameter("dcol", [128, NBT], F32, isOutput=False)
    vcol_p = nc.declare_dram_parameter("vcol", [128, NBT], F32, isOutput=False)
    # whalf = -(a/2) as a row [1, B]
    wrow_p = nc.declare_dram_parameter("wrow", [1, B], F32, isOutput=False)
    out = nc.declare_dram_parameter("out", [B, D], F32, isOutput=True)
    if DEBUG:
        dbg_m = nc.declare_dram_parameter("dbg_m", [128, NBT], F32, isOutput=True)
        dbg_l = nc.declare_dram_parameter("dbg_l", [128, NBT], F32, isOutput=True)
        dbg_acc = nc.declare_dram_parameter("dbg_acc", [128, D], F32, isOutput=True)
        dbg_pl = nc.declare_dram_parameter("dbg_pl", [128, NTILE], F32, isOutput=True)

    ds_t = ds.ap().rearrange("(t j p) d -> t j p d", j=4, p=128)  # [nt, 4, 128, 512]
    xt_t = xt.ap().rearrange("(i p) d -> i p d", p=128)  # [8, 128, 512]
    out_t = out.ap().rearrange("(i p) d -> i p d", p=128)

    with tile.TileContext(nc, linearize=LINEARIZE) as tc:
        with (
            tc.tile_pool(name="persist", bufs=1) as pp,
            tc.tile_pool(name="xn", bufs=2) as xnp,
            tc.tile_pool(name="nat", bufs=BUFS_NAT) as natp,
            tc.tile_pool(name="natbf", bufs=BUFS_NAT) as natbfp,
            tc.tile_pool(name="dt", bufs=BUFS_DT) as dtp,
            tc.tile_pool(name="sq", bufs=BUFS_SQ) as sqp,
            tc.tile_pool(name="res", bufs=2) as resp,
            tc.tile_pool(name="sf", bufs=BUFS_SF) as sfp,
            tc.tile_pool(name="dn", bufs=BUFS_DN) as dnp,
            tc.tile_pool(name="tiny", bufs=BUFS_TINY) as tp,
            tc.tile_pool(name="fin", bufs=2) as finp,
            tc.tile_pool(name="psL", bufs=BUFS_PSL, space="PSUM") as psL,
            tc.tile_pool(name="psA", bufs=BUFS_PSA, space="PSUM") as psA,
            tc.tile_pool(name="psT", bufs=BUFS_PST, space="PSUM") as psT,
            tc.tile_pool(name="dram", bufs=1, space="DRAM") as dram,
        ):
            # ---------------- constants / setup ----------------
            ident = pp.tile([128, 128], F32)
            make_identity(nc, ident[:])
            ident_bf = pp.tile([128, 128], BF16)
            nc.vector.tensor_copy(ident_bf[:], ident[:])
            ones_f = pp.tile([128, 1], F32)
            nc.vector.memset(ones_f[:], 1.0)

            ucol = pp.tile([128, NBT], F32)
            nucol = pp.tile([128, NBT], F32)
            dcol = pp.tile([128, NBT], F32)
            vcol = pp.tile([128, NBT], F32)
            for t_, p_ in ((ucol, ucol_p), (nucol, nucol_p), (dcol, dcol_p), (vcol, vcol_p)):
                nc.sync.dma_start(out=t_[:], in_=p_.ap())

            wrow = pp.tile([1, B], F32)
            nc.sync.dma_start(out=wrow[:], in_=wrow_p.ap())
            if MM1_MODE == "split":
                # whalf hi/lo f32r rows -> w3 [3, B] = (wh, wh, wl)
                w3 = pp.tile([3, B], F32R)
                wh = pp.tile([1, B], F32R)
                wres = pp.tile([1, B], F32)
                wl = pp.tile([1, B], F32R)
                nc.vector.tensor_copy(wh[:], wrow[:])
                nc.vector.tensor_tensor(wres[:], wrow[:], wh[:], op=OP.subtract)
                nc.vector.tensor_copy(wl[:], wres[:])
                nc.sync.dma_start(out=w3[0:1, :], in_=wh[:])
                nc.sync.dma_start(out=w3[1:2, :], in_=wh[:])
                nc.sync.dma_start(out=w3[2:3, :], in_=wl[:])
            elif MM1_MODE == "bsplit":
                # 3-way bf16 split of whalf: rows (w1,w1,w1,w2,w2,w3)
                w3 = pp.tile([6, B], BF16)
                wsp = [pp.tile([1, B], BF16, name=f"wsp{j}") for j in range(3)]
                wr1 = pp.tile([1, B], F32)
                wr2 = pp.tile([1, B], F32)
                nc.vector.tensor_copy(wsp[0][:], wrow[:])
                nc.vector.tensor_tensor(wr1[:], wrow[:], wsp[0][:], op=OP.subtract)
                nc.vector.tensor_copy(wsp[1][:], wr1[:])
                nc.vector.tensor_tensor(wr2[:], wr1[:], wsp[1][:], op=OP.subtract)
                nc.vector.tensor_copy(wsp[2][:], wr2[:])
                for r, j in enumerate((0, 0, 0, 1, 1, 2)):
                    nc.sync.dma_start(out=w3[r:r + 1, :], in_=wsp[j][:])

            # x_tT (+ residual split): [4][128, B]
            xdt = SDT if split else F32
            xh = [pp.tile([128, B], xdt, tag=f"xh{k}", name=f"xh{k}") for k in range(4)]
            if split:
                xl = [pp.tile([128, B], SDT, tag=f"xl{k}", name=f"xl{k}") for k in range(4)]
            for i in range(NBT):
                xnat = xnp.tile([128, D], F32, tag="xnat")
                nc.sync.dma_start(out=xnat[:], in_=xt_t[i])
                for k in range(4):
                    pX = psT.tile([128, 128], F32, tag="pT", name="pX")
                    nc.tensor.transpose(pX[:], xnat[:, k * 128:(k + 1) * 128], ident[:])
                    sl = (slice(None), slice(i * 128, (i + 1) * 128))
                    nc.scalar.copy(xh[k][sl], pX[:])
                    if split:
                        rX = resp.tile([128, 128], F32, tag="rX")
                        nc.vector.tensor_tensor(rX[:], pX[:], xh[k][sl], op=OP.subtract)
                        nc.vector.tensor_copy(xl[k][sl], rX[:])

            # running stats
            m_run = pp.tile([128, NBT], F32)
            l_run = pp.tile([128, NBT], F32)
            acc = [pp.tile([128, D], F32R, tag=f"acc{i}", name=f"acc{i}") for i in range(NBT)]
            nc.vector.memset(m_run[:], -1.0e30)
            nc.vector.memset(l_run[:], 0.0)
            for i in range(NBT):
                nc.vector.memset(acc[i][:].bitcast(F32), 0.0)

            dn_dram = dram.tile([2, n_sh], F32R)

            # ---------------- main loop over dataset tiles ----------------
            for t in range(n_tiles):
                nat = [natp.tile([128, D], F32, tag=f"nat{j}", name=f"nat{j}") for j in range(4)]
                natbf = [natbfp.tile([128, D], BF16, tag=f"natbf{j}", name=f"natbf{j}") for j in range(4)]
                for j in range(4):
                    nc.sync.dma_start(out=nat[j][:], in_=ds_t[t, j])
                    if "natbf" not in SIM_SKIP:
                        nc.gpsimd.tensor_copy(natbf[j][:], nat[j][:])

                # transposed dataset chunks dT* [4][128d, 512n], and
                # dn row = sum_d dataT^2 via fp32 ones-matmul on Square(dataT)
                ddt = SDT if split else F32
                pD = psT.tile([1, NTILE], F32, tag="pT", name="pD")
                dTh = [dtp.tile([128, NTILE], ddt, tag=f"dTh{k}", name=f"dTh{k}") for k in range(4)]
                if split:
                    dTl = [dtp.tile([128, NTILE], SDT, tag=f"dTl{k}", name=f"dTl{k}") for k in range(4)]
                for k in range(4):
                    pT = psT.tile([128, NTILE], F32, tag="pT")
                    if "dtr" not in SIM_SKIP:
                        for j in range(4):
                            nc.tensor.transpose(
                                pT[:, j * 128:(j + 1) * 128],
                                nat[j][:, k * 128:(k + 1) * 128],
                                ident[:],
                            )
                    if "dtcast" not in SIM_SKIP:
                        nc.scalar.copy(dTh[k][:], pT[:])
                        if split:
                            rT = resp.tile([128, NTILE], F32, tag="rT")
                            nc.vector.tensor_tensor(rT[:], pT[:], dTh[k][:], op=OP.subtract)
                            nc.vector.tensor_copy(dTl[k][:], rT[:])
                    if "dn" not in SIM_SKIP:
                        sq = sqp.tile([128, D], F32, tag="sq")
                        nc.scalar.activation(sq[:], pT[:], AF.Square)
                        nc.tensor.matmul(
                            pD[:], ones_f[:], sq[:], start=(k == 0), stop=(k == 3)
                        )
                sl_n = slice(t * NTILE, (t + 1) * NTILE)
                if MM1_MODE == "bsplit":
                    dnf = dnp.tile([1, NTILE], F32, tag="dnf")
                    nc.scalar.copy(dnf[:], pD[:])
                    d1 = dnp.tile([1, NTILE], BF16, tag="d1")
                    d2 = dnp.tile([1, NTILE], BF16, tag="d2")
                    d3 = dnp.tile([1, NTILE], BF16, tag="d3")
                    r1 = dnp.tile([1, NTILE], F32, tag="r1")
                    r2 = dnp.tile([1, NTILE], F32, tag="r2")
                    nc.vector.tensor_copy(d1[:], dnf[:])
                    nc.vector.tensor_tensor(r1[:], dnf[:], d1[:], op=OP.subtract)
                    nc.vector.tensor_copy(d2[:], r1[:])
                    nc.vector.tensor_tensor(r2[:], r1[:], d2[:], op=OP.subtract)
                    nc.vector.tensor_copy(d3[:], r2[:])
                    # dnK rows = (dn1,dn2,dn3,dn1,dn2,dn1) via direct SBUF DMAs
                    dnK = dnp.tile([6, NTILE], BF16, tag="dnK")
                    for r, src in enumerate((d1, d2, d3, d1, d2, d1)):
                        nc.sync.dma_start(out=dnK[r:r + 1, :], in_=src[:])
                elif split:
                    dnh_row = dnp.tile([1, NTILE], F32R, tag="dnh_row")
                    dnr_row = dnp.tile([1, NTILE], F32, tag="dnr_row")
                    dnl_row = dnp.tile([1, NTILE], F32R, tag="dnl_row")
                    nc.scalar.copy(dnh_row[:], pD[:])
                    nc.vector.tensor_tensor(dnr_row[:], pD[:], dnh_row[:], op=OP.subtract)
                    nc.vector.tensor_copy(dnl_row[:], dnr_row[:])
                    nc.sync.dma_start(out=dn_dram[0, sl_n], in_=dnh_row[:])
                    nc.sync.dma_start(out=dn_dram[1, sl_n], in_=dnl_row[:])
                    dnK = dnp.tile([3, NTILE], F32R, tag="dnK")
                    nc.sync.dma_start(out=dnK[0:2, :], in_=dn_dram[:, sl_n])
                    nc.sync.dma_start(out=dnK[2:3, :], in_=dn_dram[0:1, sl_n])
                else:
                    dnh_row = dnp.tile([1, NTILE], F32, tag="dnh_row")
                    nc.scalar.copy(dnh_row[:], pD[:])
                    nc.sync.dma_start(out=dn_dram[0, sl_n].bitcast(F32), in_=dnh_row[:])
                    dnK = dnp.tile([1, NTILE], F32, tag="dnK")
                    nc.sync.dma_start(out=dnK[:], in_=dn_dram[0, sl_n].bitcast(F32))

                # per b-tile: MM1, online softmax, MM2
                for i in range(NBT):
                    bi = slice(i * 128, (i + 1) * 128)
                    pL = psL.tile([128, NTILE], F32, tag="pL")
                    first = True
                    passes = ((xh, dTh), (xh, dTl), (xl, dTh)) if split else ((xh, dTh),)
                    if "mm1" not in SIM_SKIP:
                        for hk, dk in passes:
                            for k in range(4):
                                nc.tensor.matmul(
                                    pL[:], hk[k][:, bi], dk[k][:],
                                    start=first, stop=False,
                                )
                                first = False
                    wK = w3 if split else wrow
                    nc.tensor.matmul(pL[:], wK[:, bi], dnK[:], start=first, stop=True)
                    if DEBUG and t == 0 and i == 0:
                        plc = finp.tile([128, NTILE], F32, tag="accs", name="plc")
                        nc.scalar.copy(plc[:], pL[:])
                        nc.sync.dma_start(out=dbg_pl.ap(), in_=plc[:])

                    # online max update
                    if "stats" in SIM_SKIP:
                        continue
                    mt = tp.tile([128, 1], F32, tag="mt")
                    nc.vector.tensor_reduce(mt[:], pL[:], axis=AX.X, op=OP.max)
                    dlt = tp.tile([128, 1], F32, tag="dlt")
                    # dlt = min(m_old - mt, 0) = m_old - m_new
                    nc.vector.tensor_scalar(
                        out=dlt[:], in0=m_run[:, i:i + 1], scalar1=mt[:],
                        scalar2=0.0, op0=OP.subtract, op1=OP.min,
                    )
                    nc.vector.tensor_tensor(
                        m_run[:, i:i + 1], m_run[:, i:i + 1], mt[:], op=OP.max
                    )
                    alpha = tp.tile([128, 1], F32, tag="alpha")
                    nc.scalar.activation(
                        alpha[:], dlt[:], AF.Exp, bias=0.0, scale=ucol[:, i:i + 1]
                    )
                    # bias = -u * m_new
                    ebias = tp.tile([128, 1], F32, tag="ebias")
                    nc.vector.tensor_tensor(
                        ebias[:], nucol[:, i:i + 1], m_run[:, i:i + 1], op=OP.mult
                    )
                    # prob = exp(u*P + bias), lt = rowsum
                    prob = sfp.tile([128, NTILE], BF16, tag="prob")
                    lt = tp.tile([128, 1], F32, tag="lt")
                    nc.scalar.activation(
                        prob[:], pL[:], AF.Exp,
                        bias=ebias[:], scale=ucol[:, i:i + 1], accum_out=lt[:],
                    )
                    # l = l*alpha + lt (fused)
                    nc.vector.scalar_tensor_tensor(
                        out=l_run[:, i:i + 1], in0=l_run[:, i:i + 1],
                        scalar=alpha[:], in1=lt[:], op0=OP.mult, op1=OP.add,
                    )
                    # probT transpose (bf16): xbar DMA or PE fallback
                    if "tail" in SIM_SKIP:
                        continue
                    probT = sfp.tile([128, NTILE], BF16, tag="probT")
                    if USE_XBAR:
                        for k in range(4):
                            ksl = slice(k * 128, (k + 1) * 128)
                            nc.sync.dma_start_transpose(probT[:, ksl], prob[:, ksl])
                    else:
                        pP = psA.tile([128, NTILE], BF16, tag="pA", name="pP")
                        for k in range(4):
                            ksl = slice(k * 128, (k + 1) * 128)
                            nc.tensor.transpose(pP[:, ksl], prob[:, ksl], ident_bf[:])
                        nc.scalar.copy(probT[:], pP[:])
                    # diag(alpha) as f32r
                    diag = sfp.tile([128, 128], F32R, tag="diag")
                    nc.vector.tensor_scalar(
                        out=diag[:], in0=ident[:], scalar1=alpha[:],
                        scalar2=None, op0=OP.mult,
                    )
                    # MM2: acc_new = diag @ acc + probT-chunks @ natbf
                    pA = psA.tile([128, D], F32, tag="pA")
                    nc.tensor.matmul(pA[:], diag[:], acc[i][:], start=True, stop=False)
                    for k in range(4):
                        ksl = slice(k * 128, (k + 1) * 128)
                        nc.tensor.matmul(
                            pA[:], probT[:, ksl], natbf[k][:],
                            start=False, stop=(k == 3),
                        )
                    if ACC_COPY_DVE:
                        nc.vector.tensor_copy(acc[i][:], pA[:])
                    else:
                        nc.scalar.copy(acc[i][:], pA[:])

            if DEBUG:
                nc.sync.dma_start(out=dbg_m.ap(), in_=m_run[:])
                nc.sync.dma_start(out=dbg_l.ap(), in_=l_run[:])
                acc0c = finp.tile([128, D], F32, tag="accs", name="acc0c")
                nc.vector.tensor_copy(acc0c[:], acc[0][:])
                nc.sync.dma_start(out=dbg_acc.ap(), in_=acc0c[:])

            # ---------------- cross-core merge ----------------
            m_cc_in = dram.tile([128, NBT], F32)
            m_cc_out = dram.tile([128, NBT], F32)
            nc.sync.dma_start(out=m_cc_in[:], in_=m_run[:])
            if not SIM_1CORE:
                nc.gpsimd.collective_compute(
                    "AllReduce", OP.max,
                    replica_groups=[list(range(NCORES))],
                    ins=[m_cc_in[:].opt()], outs=[m_cc_out[:].opt()],
                )
            else:
                nc.sync.dma_start(out=m_cc_out[:], in_=m_cc_in[:])
            m_glob = pp.tile([128, NBT], F32)
            nc.sync.dma_start(out=m_glob[:], in_=m_cc_out[:])

            # gamma_i = exp(u * (m_loc - m_glob)); scale acc, l
            dg = pp.tile([128, NBT], F32)
            nc.vector.tensor_tensor(dg[:], m_run[:], m_glob[:], op=OP.subtract)
            gam = pp.tile([128, NBT], F32)
            for i in range(NBT):
                nc.scalar.activation(
                    gam[:, i:i + 1], dg[:, i:i + 1], AF.Exp,
                    bias=0.0, scale=ucol[:, i:i + 1],
                )
            nc.vector.tensor_tensor(l_run[:], l_run[:], gam[:], op=OP.mult)

            accl_in = dram.tile([128, NBT * D + NBT], F32)
            accl_out = dram.tile([128, NBT * D + NBT], F32)
            for i in range(NBT):
                accs = finp.tile([128, D], F32, tag="accs")
                nc.vector.tensor_scalar(
                    out=accs[:], in0=acc[i][:], scalar1=gam[:, i:i + 1],
                    scalar2=None, op0=OP.mult,
                )
                nc.sync.dma_start(out=accl_in[:, i * D:(i + 1) * D], in_=accs[:])
            nc.sync.dma_start(out=accl_in[:, NBT * D:], in_=l_run[:])
            if not SIM_1CORE:
                nc.gpsimd.collective_compute(
                    "AllReduce", OP.add,
                    replica_groups=[list(range(NCORES))],
                    ins=[accl_in[:].opt()], outs=[accl_out[:].opt()],
                )
            else:
                nc.sync.dma_start(out=accl_out[:], in_=accl_in[:])

            lg = pp.tile([128, NBT], F32)
            nc.sync.dma_start(out=lg[:], in_=accl_out[:, NBT * D:])
            rl = pp.tile([128, NBT], F32)
            nc.vector.reciprocal(rl[:], lg[:])
            # s1 = dcoef / l
            s1 = pp.tile([128, NBT], F32)
            nc.vector.tensor_tensor(s1[:], dcol[:], rl[:], op=OP.mult)
            for i in range(NBT):
                accg = finp.tile([128, D], F32, tag="accg")
                nc.sync.dma_start(out=accg[:], in_=accl_out[:, i * D:(i + 1) * D])
                xnat = xnp.tile([128, D], F32, tag="xnat")
                nc.sync.dma_start(out=xnat[:], in_=xt_t[i])
                v1 = finp.tile([128, D], F32, tag="v1")
                nc.vector.tensor_scalar(
                    out=v1[:], in0=accg[:], scalar1=s1[:, i:i + 1],
                    scalar2=None, op0=OP.mult,
                )
                v2 = finp.tile([128, D], F32, tag="v2")
                nc.vector.tensor_scalar(
                    out=v2[:], in0=xnat[:], scalar1=vcol[:, i:i + 1],
                    scalar2=None, op0=OP.mult,
                )
                nc.vector.tensor_tensor(v1[:], v1[:], v2[:], op=OP.add)
                nc.sync.dma_start(out=out_t[i], in_=v1[:])

    nc.compile()
    return nc


_BUILD_CACHE = {}


def _get_nc(n_tiles):
    key = (n_tiles, MM1_MODE, USE_XBAR, LINEARIZE, DEBUG, SIM_1CORE, BUFS_NAT, BUFS_DT, BUFS_SF, BUFS_DN, BUFS_TINY, ACC_COPY_DVE, BUFS_PSL, BUFS_PSA, BUFS_PST, BUFS_SQ)
    if key not in _BUILD_CACHE:
        _BUILD_CACHE[key] = build(n_tiles)
    return _BUILD_CACHE[key]


def make_in_maps(x_t, t, dataset, n_tiles):
    """Shard + pad dataset, compute coefficient vectors."""
    n = dataset.shape[0]
    n_pad = NCORES * n_tiles * NTILE
    assert n_pad >= n
    dpad = np.zeros((n_pad, D), dtype=np.float32)
    dpad[:n] = dataset
    dpad[n:, 0] = 1000.0  # far-away pad rows: huge norm, ~zero softmax weight
    shards = dpad.reshape(NCORES, n_tiles * NTILE, D)

    a = t.astype(np.float64)
    b = 1.0 - a
    u = (a / (b * b)).astype(np.float32)
    w = (-a / 2.0).astype(np.float32)
    dcoef = (1.0 + a / b).astype(np.float32)
    vcoef = (-1.0 / b).astype(np.float32)

    def col(v):
        return np.ascontiguousarray(v.reshape(NBT, 128).T)

    base = dict(
        x_t=np.ascontiguousarray(x_t),
        ucol=col(u),
        nucol=col(-u),
        dcol=col(dcoef),
        vcol=col(vcoef),
        wrow=np.ascontiguousarray(w.reshape(1, B)),
    )
    return [dict(base, dataset=np.ascontiguousarray(shards[c])) for c in range(NCORES)]


def kernel(x_t, t, dataset):
    x_t = np.asarray(x_t, dtype=np.float32)
    t = np.asarray(t, dtype=np.float32)
    dataset = np.asarray(dataset, dtype=np.float32)
    n = dataset.shape[0]
    n_tiles = -(-n // (NCORES * NTILE))  # ceil -> 25 for N=100000
    nc = _get_nc(n_tiles)
    in_maps = make_in_maps(x_t, t, dataset, n_tiles)
    res = run_bass_kernel_spmd(nc, in_maps, core_ids=list(range(NCORES)))
    return np.asarray(res.results[0]["out"], dtype=np.float32)


def ref_numpy(x_t, t, dataset):
    aa = t.astype(np.float64)
    bb = 1.0 - aa
    dsn = (dataset.astype(np.float64) ** 2).sum(1)
    t2 = x_t.astype(np.float64) @ dataset.T.astype(np.float64)
    logit = (-1.0 / (2 * bb * bb))[:, None] * (
        (x_t.astype(np.float64) ** 2).sum(1)[:, None]
        - 2 * aa[:, None] * t2
        + (aa * aa)[:, None] * dsn[None, :]
    )
    p = np.exp(logit - logit.max(1, keepdims=True))
    p /= p.sum(1, keepdims=True)
    p = p * (1 + aa / bb)[:, None]
    return (-1.0 / bb)[:, None] * x_t.astype(np.float64) + p @ dataset.astype(np.float64)


if __name__ == "__main__":
    rng = np.random.default_rng(0)
    n = 2 * NCORES * NTILE - 300
    x_t = rng.standard_normal((B, D)).astype(np.float32)
    t = rng.uniform(0.05, 0.95, (B,)).astype(np.float32)
    dataset = rng.standard_normal((n, D)).astype(np.float32)
    v = kernel(x_t, t, dataset)
    vref = ref_numpy(x_t, t, dataset)
    err = np.linalg.norm(v - vref) / np.linalg.norm(vref)
    print("rel l2 err:", err)
    print("max abs err:", np.abs(v - vref).max(), "ref scale:", np.abs(vref).max())



# revision 3
# speedup vs baseline: 885.9559x; 885.9559x over previous
"""AnalyticGaussianVelocity (soft-kNN flow velocity) on 8 trn2 NeuronCores.

Math (reference):
    a = t, b = 1-t
    logit[b,n] = -1/(2 b^2) * ||x_b - a * d_n||^2
    prob = softmax(logit, axis=n) * (1 + a/b)
    v = (-1/b) x + prob @ dataset

Dropping per-row constants, softmax(logit) == softmax(u * P) with
    u = a/b^2  (>0),  P[b,n] = x_b . d_n - (a/2) ||d_n||^2

Kernel strategy (v2):
  - dataset sharded over N across 8 cores; per-core flash-style online
    softmax; partial (m, l, acc) returned per core and merged on HOST
    (no collectives on device).
  - all layout work on host: dataset pre-transposed and pre-split into
    f32r-exact 11-bit hi/lo components (f32r matmul is exact for 11-bit
    mantissas and runs 1 cyc/row like bf16); norms and -(a/2) rows
    pre-split 2-way 11-bit (K=4 exact norm matmul).
  - rows sorted by t on host so that precision tiers align with b-tiles:
    logit abs error tolerance is ~0.1/u with u = a/(1-a)^2, so low-u
    b-tiles need only the single hi*hi matmul pass (12-bit operands,
    eps ~ 3e-3), while high-u b-tiles use 3 passes (hi*hi + hi*lo +
    lo*hi, eps ~ 6e-7). Pass count chosen per b-tile from its max u.
  - MM2: prob (bf16 from ACT exp) -> PE bf16 transposes -> probT @
    dataset_bf16; accumulator update acc = alpha*acc + pA fused on DVE
    reading PSUM directly (no diag-rescale matmul, no ACT acc copy).
"""

import sys

sys.path.insert(0, "/opt/trn_rl_repo")

import numpy as np
import ml_dtypes

import concourse.bass as bass
import concourse.mybir as mybir
import concourse.tile as tile
from concourse import bacc
from concourse.bass_utils import run_bass_kernel_spmd
from concourse.masks import make_identity

B, D = 1024, 512
NCORES = 8
NTILE = 512  # dataset rows per n-tile
NBT = B // 128  # 8 b-tiles

F32 = mybir.dt.float32
F32R = mybir.dt.float32r
BF16 = mybir.dt.bfloat16

AF = mybir.ActivationFunctionType
OP = mybir.AluOpType
AX = mybir.AxisListType

# u threshold below which a single f32r hi*hi pass is accurate enough
U_1PASS = 12.0


def build(n_tiles, tiers, reps=1, ndev=NCORES):
    """tiers: tuple of NBT ints in {1,3} — MM1 passes per sorted b-tile."""
    n_sh = n_tiles * NTILE
    nc = bacc.Bacc("TRN2", target_bir_lowering=False, debug=False, num_devices=ndev)

    # --- dram params (per core) ---
    # transposed dataset hi/lo, layout [t, k(128-chunk of d), 128 d, NTILE n]
    dsth_p = nc.declare_dram_parameter("dsth", [n_tiles * 4 * 128, NTILE], F32R, isOutput=False)
    dstl_p = nc.declare_dram_parameter("dstl", [n_tiles * 4 * 128, NTILE], F32R, isOutput=False)
    # row-major bf16 dataset for MM2, layout [t, j(128-chunk of n), 128 n, D]
    natb_p = nc.declare_dram_parameter("natb", [n_tiles * 4 * 128, D], BF16, isOutput=False)
    # norm rows (wh.dnh + wh.dnl + wl.dnh + wl.dnl): dn4 = (dnh,dnl,dnh,dnl)
    dn4_p = nc.declare_dram_parameter("dn4", [4, n_sh], F32R, isOutput=False)
    # w4 = (wh,wh,wl,wl) where w = -(a/2) per sorted b row, [4, B]
    w4_p = nc.declare_dram_parameter("w4", [4, B], F32R, isOutput=False)
    # x^T hi/lo splits, layout [k(128-chunk of d), 128 d, B]
    xh_p = nc.declare_dram_parameter("xh", [4 * 128, B], F32R, isOutput=False)
    xl_p = nc.declare_dram_parameter("xl", [4 * 128, B], F32R, isOutput=False)
    # per-b coefficient columns [128, NBT]: col i holds b = i*128+p (sorted order)
    ucol_p = nc.declare_dram_parameter("ucol", [128, NBT], F32, isOutput=False)
    nucol_p = nc.declare_dram_parameter("nucol", [128, NBT], F32, isOutput=False)
    # outputs: per-core partial softmax state
    m_out = nc.declare_dram_parameter("m_out", [128, NBT], F32, isOutput=True)
    l_out = nc.declare_dram_parameter("l_out", [128, NBT], F32, isOutput=True)
    acc_out = nc.declare_dram_parameter("acc_out", [NBT * 128, D], F32, isOutput=True)

    dsth_t = dsth_p.ap().rearrange("(t k p) n -> t k p n", k=4, p=128)
    dstl_t = dstl_p.ap().rearrange("(t k p) n -> t k p n", k=4, p=128)
    natb_t = natb_p.ap().rearrange("(t j p) d -> t j p d", j=4, p=128)
    xh_t = xh_p.ap().rearrange("(k p) b -> k p b", p=128)
    xl_t = xl_p.ap().rearrange("(k p) b -> k p b", p=128)
    acc_out_t = acc_out.ap().rearrange("(i p) d -> i p d", p=128)

    with tile.TileContext(nc) as tc:
        with (
            tc.tile_pool(name="persist", bufs=1) as pp,
            tc.tile_pool(name="dt", bufs=2) as dtp,
            tc.tile_pool(name="nat", bufs=2) as natp,
            tc.tile_pool(name="sf", bufs=4) as sfp,
            tc.tile_pool(name="tiny", bufs=6) as tp,
            tc.tile_pool(name="psL", bufs=3, space="PSUM") as psL,
            tc.tile_pool(name="psA", bufs=2, space="PSUM") as psA,
            tc.tile_pool(name="psT", bufs=2, space="PSUM") as psT,
        ):
            # ---------------- resident setup ----------------
            ident = pp.tile([128, 128], F32)
            make_identity(nc, ident[:])
            ident_bf = pp.tile([128, 128], BF16)
            nc.vector.tensor_copy(ident_bf[:], ident[:])

            xh_s = [pp.tile([128, B], F32R, name=f"xh{k}") for k in range(4)]
            xl_s = [pp.tile([128, B], F32R, name=f"xl{k}") for k in range(4)]
            for k in range(4):
                nc.sync.dma_start(out=xh_s[k][:], in_=xh_t[k])
                nc.sync.dma_start(out=xl_s[k][:], in_=xl_t[k])
            w4_s = pp.tile([4, B], F32R)
            nc.sync.dma_start(out=w4_s[:], in_=w4_p.ap())
            dn4_s = pp.tile([4, n_sh], F32R)
            nc.sync.dma_start(out=dn4_s[:], in_=dn4_p.ap())
            ucol = pp.tile([128, NBT], F32)
            nucol = pp.tile([128, NBT], F32)
            nc.sync.dma_start(out=ucol[:], in_=ucol_p.ap())
            nc.sync.dma_start(out=nucol[:], in_=nucol_p.ap())

            m_run = pp.tile([128, NBT], F32)
            l_run = pp.tile([128, NBT], F32)
            acc = [pp.tile([128, D], F32, name=f"acc{i}") for i in range(NBT)]

            for _rep in range(reps):
                nc.vector.memset(m_run[:], -1.0e30)
                nc.vector.memset(l_run[:], 0.0)
                for i in range(NBT):
                    nc.vector.memset(acc[i][:], 0.0)

                # ---------------- main loop over dataset tiles ----------------
                for t in range(n_tiles):
                    dTh = [dtp.tile([128, NTILE], F32R, tag=f"dTh{k}", name=f"dTh{k}") for k in range(4)]
                    dTl = [dtp.tile([128, NTILE], F32R, tag=f"dTl{k}", name=f"dTl{k}") for k in range(4)]
                    nb = [natp.tile([128, D], BF16, tag=f"nb{j}", name=f"nb{j}") for j in range(4)]
                    for k in range(4):
                        nc.sync.dma_start(out=dTh[k][:], in_=dsth_t[t, k])
                        nc.sync.dma_start(out=dTl[k][:], in_=dstl_t[t, k])
                        nc.sync.dma_start(out=nb[k][:], in_=natb_t[t, k])
                    sl_n = slice(t * NTILE, (t + 1) * NTILE)

                    for i in range(NBT):
                        bi = slice(i * 128, (i + 1) * 128)
                        pL = psL.tile([128, NTILE], F32, tag="pL")
                        passes = ((xh_s, dTh), (xh_s, dTl), (xl_s, dTh))[: tiers[i]]
                        first = True
                        for hk, dk in passes:
                            for k in range(4):
                                nc.tensor.matmul(
                                    pL[:], hk[k][:, bi], dk[k][:],
                                    start=first, stop=False,
                                )
                                first = False
                        nc.tensor.matmul(
                            pL[:], w4_s[:, bi], dn4_s[:, sl_n], start=first, stop=True
                        )

                        # online softmax stats
                        mt = tp.tile([128, 1], F32, tag="mt")
                        nc.vector.tensor_reduce(mt[:], pL[:], axis=AX.X, op=OP.max)
                        dlt = tp.tile([128, 1], F32, tag="dlt")
                        # dlt = min(m_old - mt, 0) = m_old - m_new
                        nc.vector.tensor_scalar(
                            out=dlt[:], in0=m_run[:, i:i + 1], scalar1=mt[:],
                            scalar2=0.0, op0=OP.subtract, op1=OP.min,
                        )
                        nc.vector.tensor_tensor(
                            m_run[:, i:i + 1], m_run[:, i:i + 1], mt[:], op=OP.max
                        )
                        alpha = tp.tile([128, 1], F32, tag="alpha")
                        nc.scalar.activation(
                            alpha[:], dlt[:], AF.Exp, bias=0.0, scale=ucol[:, i:i + 1]
                        )
                        ebias = tp.tile([128, 1], F32, tag="ebias")
                        nc.vector.tensor_tensor(
                            ebias[:], nucol[:, i:i + 1], m_run[:, i:i + 1], op=OP.mult
                        )
                        # prob = exp(u*P + bias) in bf16, lt = rowsum
                        prob = sfp.tile([128, NTILE], BF16, tag="prob")
                        lt = tp.tile([128, 1], F32, tag="lt")
                        nc.scalar.activation(
                            prob[:], pL[:], AF.Exp,
                            bias=ebias[:], scale=ucol[:, i:i + 1], accum_out=lt[:],
                        )
                        # l = l*alpha + lt (fused DVE)
                        nc.vector.scalar_tensor_tensor(
                            out=l_run[:, i:i + 1], in0=l_run[:, i:i + 1],
                            scalar=alpha[:], in1=lt[:], op0=OP.mult, op1=OP.add,
                        )
                        # probT via PE bf16 transposes
                        pP = psT.tile([128, NTILE], BF16, tag="pP")
                        for k in range(4):
                            ksl = slice(k * 128, (k + 1) * 128)
                            nc.tensor.transpose(pP[:, ksl], prob[:, ksl], ident_bf[:])
                        probT = sfp.tile([128, NTILE], BF16, tag="probT")
                        nc.scalar.copy(probT[:], pP[:])
                        # MM2: pA = probT-chunks @ nb
                        pA = psA.tile([128, D], F32, tag="pA")
                        for k in range(4):
                            ksl = slice(k * 128, (k + 1) * 128)
                            nc.tensor.matmul(
                                pA[:], probT[:, ksl], nb[k][:],
                                start=(k == 0), stop=(k == 3),
                            )
                        # acc = alpha*acc + pA (fused DVE, reads PSUM)
                        nc.vector.scalar_tensor_tensor(
                            out=acc[i][:], in0=acc[i][:],
                            scalar=alpha[:], in1=pA[:], op0=OP.mult, op1=OP.add,
                        )

                # ---------------- write partial state ----------------
                nc.sync.dma_start(out=m_out.ap(), in_=m_run[:])
                nc.sync.dma_start(out=l_out.ap(), in_=l_run[:])
                for i in range(NBT):
                    nc.sync.dma_start(out=acc_out_t[i], in_=acc[i][:])

    nc.compile()
    return nc


_BUILD_CACHE = {}


def _get_nc(n_tiles, tiers, reps=1, ndev=NCORES):
    key = (n_tiles, tuple(tiers), reps, ndev)
    if key not in _BUILD_CACHE:
        _BUILD_CACHE[key] = build(n_tiles, tuple(tiers), reps=reps, ndev=ndev)
    return _BUILD_CACHE[key]


def _rne11(x):
    """Round fp32 to 11 explicit mantissa bits (f32r-exact), round-half-even."""
    xi = np.ascontiguousarray(x, dtype=np.float32).view(np.uint32)
    keep = np.uint32(0xFFFFF000)
    half = np.uint32(0x800)
    odd = (xi >> np.uint32(12)) & np.uint32(1)
    r = (xi + (half - np.uint32(1)) + odd) & keep
    return r.view(np.float32)


def _trunc11(x):
    xi = np.ascontiguousarray(x, dtype=np.float32).view(np.uint32)
    return (xi & np.uint32(0xFFFFF000)).view(np.float32)


def _split11(x):
    hi = _rne11(x)
    lo = _trunc11((x.astype(np.float32) - hi))
    return hi, lo


def prepare(x_t, t, dataset, n_tiles):
    """Host-side layout: sort rows by t, pad+shard dataset, pre-split."""
    bf16 = ml_dtypes.bfloat16
    n = dataset.shape[0]
    n_pad = NCORES * n_tiles * NTILE
    assert n_pad >= n

    perm = np.argsort(t, kind="stable")
    xs = np.ascontiguousarray(x_t[perm])
    ts = t[perm].astype(np.float64)

    a = ts
    b = 1.0 - a
    u = (a / (b * b)).astype(np.float32)
    w = (-a / 2.0).astype(np.float32)
    dcoef = (1.0 + a / b)
    vcoef = (-1.0 / b)

    # per-b-tile pass tiers from max u in tile
    umax = u.reshape(NBT, 128).max(axis=1)
    tiers = tuple(1 if um <= U_1PASS else 3 for um in umax)

    # dataset: pad with far-away rows (value 0, huge norm)
    dn = np.einsum("nd,nd->n", dataset, dataset, dtype=np.float64).astype(np.float32)
    dn_pad = np.full(n_pad, 1.0e6, dtype=np.float32)
    dn_pad[:n] = dn
    dpad = np.zeros((n_pad, D), dtype=np.float32)
    dpad[:n] = dataset

    ds_hi32, ds_lo32 = _split11(dpad)
    # transposed splits: [core, t, k, p(128 d), NTILE]
    def tsplit(z):
        # z: [n_pad, D] f32 -> [core, t, k, 128, NTILE]
        zt = z.T.reshape(4, 128, NCORES, n_tiles, NTILE)
        return np.ascontiguousarray(zt.transpose(2, 3, 0, 1, 4))

    dsth = tsplit(ds_hi32)
    dstl = tsplit(ds_lo32)
    natb = np.ascontiguousarray(dpad.astype(bf16)).reshape(
        NCORES, n_tiles * 4 * 128, D
    )

    dnh, dnl = _split11(dn_pad)
    dn4 = np.stack([dnh, dnl, dnh, dnl]).reshape(4, NCORES, n_tiles * NTILE)
    wh, wl = _split11(w)
    w4 = np.ascontiguousarray(np.stack([wh, wh, wl, wl]))

    xT = xs.T.reshape(4, 128, B)
    xh, xl = _split11(xT)

    def col(v):
        return np.ascontiguousarray(v.astype(np.float32).reshape(NBT, 128).T)

    base = dict(
        xh=xh.reshape(4 * 128, B),
        xl=xl.reshape(4 * 128, B),
        w4=w4,
        ucol=col(u),
        nucol=col(-u),
    )
    in_maps = [
        dict(
            base,
            dsth=dsth[c].reshape(n_tiles * 4 * 128, NTILE),
            dstl=dstl[c].reshape(n_tiles * 4 * 128, NTILE),
            natb=natb[c],
            dn4=np.ascontiguousarray(dn4[:, c]),
        )
        for c in range(NCORES)
    ]
    aux = dict(perm=perm, u=u, dcoef=dcoef, vcoef=vcoef, xs=xs, tiers=tiers)
    return in_maps, aux


def merge(results, aux):
    """Host-side flash-softmax merge of per-core partials -> full output."""
    u = aux["u"].astype(np.float64)  # [B] sorted order
    # device layout: [128, NBT] col i holds b = i*128+p -> transpose+flatten
    def uncol(z):
        return np.asarray(z, dtype=np.float64).T.reshape(B)

    ms = np.stack([uncol(r["m_out"]) for r in results])  # [C, B]
    ls = np.stack([uncol(r["l_out"]) for r in results])  # [C, B]
    accs = np.stack([np.asarray(r["acc_out"], dtype=np.float64) for r in results])

    m_glob = ms.max(axis=0)  # [B]
    gam = np.exp(u[None, :] * (ms - m_glob[None, :]))  # [C, B]
    l_glob = (gam * ls).sum(axis=0)  # [B]
    acc_glob = np.einsum("cb,cbd->bd", gam, accs)  # [B, D]

    v_sorted = (
        aux["dcoef"][:, None] * acc_glob / l_glob[:, None]
        + aux["vcoef"][:, None] * aux["xs"].astype(np.float64)
    )
    v = np.empty((B, D), dtype=np.float32)
    v[aux["perm"]] = v_sorted.astype(np.float32)
    return v


def kernel(x_t, t, dataset):
    x_t = np.asarray(x_t, dtype=np.float32)
    t = np.asarray(t, dtype=np.float32)
    dataset = np.asarray(dataset, dtype=np.float32)
    n = dataset.shape[0]
    n_tiles = -(-n // (NCORES * NTILE))  # ceil -> 25 for N=100000
    in_maps, aux = prepare(x_t, t, dataset, n_tiles)
    nc = _get_nc(n_tiles, aux["tiers"])
    res = run_bass_kernel_spmd(nc, in_maps, core_ids=list(range(NCORES)))
    return merge(res.results, aux)


def ref_numpy(x_t, t, dataset):
    aa = t.astype(np.float64)
    bb = 1.0 - aa
    dsn = (dataset.astype(np.float64) ** 2).sum(1)
    t2 = x_t.astype(np.float64) @ dataset.T.astype(np.float64)
    logit = (-1.0 / (2 * bb * bb))[:, None] * (
        (x_t.astype(np.float64) ** 2).sum(1)[:, None]
        - 2 * aa[:, None] * t2
        + (aa * aa)[:, None] * dsn[None, :]
    )
    p = np.exp(logit - logit.max(1, keepdims=True))
    p /= p.sum(1, keepdims=True)
    p = p * (1 + aa / bb)[:, None]
    return (-1.0 / bb)[:, None] * x_t.astype(np.float64) + p @ dataset.astype(np.float64)


if __name__ == "__main__":
    rng = np.random.default_rng(0)
    n = 2 * NCORES * NTILE - 300
    x_t = rng.standard_normal((B, D)).astype(np.float32)
    t = rng.uniform(0.05, 0.95, (B,)).astype(np.float32)
    dataset = rng.standard_normal((n, D)).astype(np.float32)
    v = kernel(x_t, t, dataset)
    vref = ref_numpy(x_t, t, dataset)
    err = np.linalg.norm(v - vref) / np.linalg.norm(vref)
    print("rel l2 err:", err)
    print("max abs err:", np.abs(v - vref).max(), "ref scale:", np.abs(vref).max())


# revision 8
# speedup vs baseline: 989.8709x; 1.1173x over previous
"""AnalyticGaussianVelocity (soft-kNN flow velocity) on 8 trn2 NeuronCores.

Math (reference):
    a = t, b = 1-t
    logit[b,n] = -1/(2 b^2) * ||x_b - a * d_n||^2
    prob = softmax(logit, axis=n) * (1 + a/b)
    v = (-1/b) x + prob @ dataset

Dropping per-row constants, softmax(logit) == softmax(u * P) with
    u = a/b^2  (>0),  P[b,n] = x_b . d_n - (a/2) ||d_n||^2

Kernel strategy (v2):
  - dataset sharded over N across 8 cores; per-core flash-style online
    softmax; partial (m, l, acc) returned per core and merged on HOST
    (no collectives on device).
  - all layout work on host: dataset pre-transposed and pre-split into
    f32r-exact 11-bit hi/lo components (f32r matmul is exact for 11-bit
    mantissas and runs 1 cyc/row like bf16); norms and -(a/2) rows
    pre-split 2-way 11-bit (K=4 exact norm matmul).
  - rows sorted by t on host so that precision tiers align with b-tiles:
    logit abs error tolerance is ~0.1/u with u = a/(1-a)^2, so low-u
    b-tiles need only the single hi*hi matmul pass (12-bit operands,
    eps ~ 3e-3), while high-u b-tiles use 3 passes (hi*hi + hi*lo +
    lo*hi, eps ~ 6e-7). Pass count chosen per b-tile from its max u.
  - MM2: prob (bf16 from ACT exp) -> PE bf16 transposes -> probT @
    dataset_bf16; accumulator update acc = alpha*acc + pA fused on DVE
    reading PSUM directly (no diag-rescale matmul, no ACT acc copy).
"""

import sys

sys.path.insert(0, "/opt/trn_rl_repo")

import numpy as np
import ml_dtypes

import concourse.bass as bass
import concourse.mybir as mybir
import concourse.tile as tile
from concourse import bacc
from concourse.bass_utils import run_bass_kernel_spmd
from concourse.masks import make_identity

B, D = 1024, 512
NCORES = 8
NTILE = 512  # dataset rows per n-tile
NBT = B // 128  # 8 b-tiles

F32 = mybir.dt.float32
F32R = mybir.dt.float32r
BF16 = mybir.dt.bfloat16

AF = mybir.ActivationFunctionType
OP = mybir.AluOpType
AX = mybir.AxisListType

# u threshold below which a single f32r hi*hi pass is accurate enough:
# 1-pass logit noise is ~3e-3 rms, budget ~0.1 absolute -> u <= ~33
U_1PASS = 33.0


def build(n_tiles, tiers, reps=1, ndev=NCORES, skip=()):
    """tiers: NBT ints in {1,3} — MM1 passes per sorted b-tile.
    skip: subset of {"stats","tail","mm1"} for timing-attribution builds."""
    n_sh = n_tiles * NTILE
    nc = bacc.Bacc("TRN2", target_bir_lowering=False, debug=False, num_devices=ndev)

    # --- dram params (per core) ---
    # transposed dataset hi/lo, layout [t, k(128-chunk of d), 128 d, NTILE n]
    dsth_p = nc.declare_dram_parameter("dsth", [n_tiles * 4 * 128, NTILE], F32R, isOutput=False)
    dstl_p = nc.declare_dram_parameter("dstl", [n_tiles * 4 * 128, NTILE], F32R, isOutput=False)
    # row-major bf16 dataset for MM2, layout [t, j(128-chunk of n), 128 n, D]
    natb_p = nc.declare_dram_parameter("natb", [n_tiles * 4 * 128, D], BF16, isOutput=False)
    # norm rows (wh.dnh + wh.dnl + wl.dnh + wl.dnl): dn4 = (dnh,dnl,dnh,dnl)
    dn4_p = nc.declare_dram_parameter("dn4", [4, n_sh], F32R, isOutput=False)
    # w4 = (wh,wh,wl,wl) where w = -(a/2) per sorted b row, [4, B]
    w4_p = nc.declare_dram_parameter("w4", [4, B], F32R, isOutput=False)
    # x^T hi/lo splits, layout [k(128-chunk of d), 128 d, B]
    xh_p = nc.declare_dram_parameter("xh", [4 * 128, B], F32R, isOutput=False)
    xl_p = nc.declare_dram_parameter("xl", [4 * 128, B], F32R, isOutput=False)
    # per-b coefficient columns [128, NBT]: col i holds b = i*128+p (sorted order)
    ucol_p = nc.declare_dram_parameter("ucol", [128, NBT], F32, isOutput=False)
    nucol_p = nc.declare_dram_parameter("nucol", [128, NBT], F32, isOutput=False)
    # outputs: per-core partial softmax state
    m_out = nc.declare_dram_parameter("m_out", [128, NBT], F32, isOutput=True)
    l_out = nc.declare_dram_parameter("l_out", [128, NBT], F32, isOutput=True)
    acc_out = nc.declare_dram_parameter("acc_out", [NBT * 128, D], F32, isOutput=True)

    dsth_t = dsth_p.ap().rearrange("(t k p) n -> t k p n", k=4, p=128)
    dstl_t = dstl_p.ap().rearrange("(t k p) n -> t k p n", k=4, p=128)
    natb_t = natb_p.ap().rearrange("(t j p) d -> t j p d", j=4, p=128)
    xh_t = xh_p.ap().rearrange("(k p) b -> k p b", p=128)
    xl_t = xl_p.ap().rearrange("(k p) b -> k p b", p=128)
    acc_out_t = acc_out.ap().rearrange("(i p) d -> i p d", p=128)

    with tile.TileContext(nc) as tc:
        with (
            tc.tile_pool(name="persist", bufs=1) as pp,
            tc.tile_pool(name="dt", bufs=2) as dtp,
            tc.tile_pool(name="nat", bufs=2) as natp,
            tc.tile_pool(name="sf", bufs=4) as sfp,
            tc.tile_pool(name="tiny", bufs=6) as tp,
            tc.tile_pool(name="psL", bufs=3, space="PSUM") as psL,
            tc.tile_pool(name="psA", bufs=2, space="PSUM") as psA,
            tc.tile_pool(name="psT", bufs=2, space="PSUM") as psT,
        ):
            # ---------------- resident setup ----------------
            ident = pp.tile([128, 128], F32)
            make_identity(nc, ident[:])
            ident_bf = pp.tile([128, 128], BF16)
            nc.vector.tensor_copy(ident_bf[:], ident[:])

            xh_s = [pp.tile([128, B], F32R, name=f"xh{k}") for k in range(4)]
            xl_s = [pp.tile([128, B], F32R, name=f"xl{k}") for k in range(4)]
            for k in range(4):
                nc.sync.dma_start(out=xh_s[k][:], in_=xh_t[k])
            for k in range(4):
                nc.sync.dma_start(out=xl_s[k][:], in_=xl_t[k])
            w4_s = pp.tile([4, B], F32R)
            nc.sync.dma_start(out=w4_s[:], in_=w4_p.ap())
            dn4_s = pp.tile([4, n_sh], F32R)
            nc.sync.dma_start(out=dn4_s[:], in_=dn4_p.ap())
            ucol = pp.tile([128, NBT], F32)
            nucol = pp.tile([128, NBT], F32)
            nc.sync.dma_start(out=ucol[:], in_=ucol_p.ap())
            nc.sync.dma_start(out=nucol[:], in_=nucol_p.ap())

            m_run = pp.tile([128, NBT], F32)
            l_run = pp.tile([128, NBT], F32)
            acc = [pp.tile([128, D], F32, name=f"acc{i}") for i in range(NBT)]

            for _rep in range(reps):
                nc.vector.memset(m_run[:], -1.0e30)
                nc.vector.memset(l_run[:], 0.0)
                for i in range(NBT):
                    nc.vector.memset(acc[i][:], 0.0)
                pending = None

                # ---------------- main loop over dataset tiles ----------------
                for t in range(n_tiles):
                    dTh = [dtp.tile([128, NTILE], F32R, tag=f"dTh{k}", name=f"dTh{k}") for k in range(4)]
                    dTl = [dtp.tile([128, NTILE], F32R, tag=f"dTl{k}", name=f"dTl{k}") for k in range(4)]
                    nb = [natp.tile([128, D], BF16, tag=f"nb{j}", name=f"nb{j}") for j in range(4)]
                    for k in range(4):
                        nc.sync.dma_start(out=dTh[k][:], in_=dsth_t[t, k])
                        nc.sync.dma_start(out=dTl[k][:], in_=dstl_t[t, k])
                        nc.sync.dma_start(out=nb[k][:], in_=natb_t[t, k])
                    sl_n = slice(t * NTILE, (t + 1) * NTILE)

                    def emit_tail(i, prob, alpha, nb):
                        """PE tail of b-tile i: probT transposes + MM2, then
                        the DVE accumulator update. Emitted one b-tile late
                        so PE never waits on the softmax chain."""
                        pP = psT.tile([128, NTILE], BF16, tag="pP", name="pP")
                        for k in range(4):
                            ksl = slice(k * 128, (k + 1) * 128)
                            nc.tensor.transpose(pP[:, ksl], prob[:, ksl], ident_bf[:])
                        probT = sfp.tile([128, NTILE], BF16, tag="probT", name="probT")
                        nc.scalar.copy(probT[:], pP[:])
                        pA = psA.tile([128, D], F32, tag="pA", name="pA")
                        for k in range(4):
                            ksl = slice(k * 128, (k + 1) * 128)
                            nc.tensor.matmul(
                                pA[:], probT[:, ksl], nb[k][:],
                                start=(k == 0), stop=(k == 3),
                            )
                        # acc = alpha*acc + pA (fused DVE, reads PSUM)
                        nc.vector.scalar_tensor_tensor(
                            out=acc[i][:], in0=acc[i][:],
                            scalar=alpha[:], in1=pA[:], op0=OP.mult, op1=OP.add,
                        )

                    for i in range(NBT):
                        bi = slice(i * 128, (i + 1) * 128)
                        pL = psL.tile([128, NTILE], F32, tag="pL")
                        passes = ((xh_s, dTh), (xh_s, dTl), (xl_s, dTh))[: tiers[i]]
                        if "mm1" in skip:
                            passes = ()
                        first = True
                        for hk, dk in passes:
                            for k in range(4):
                                nc.tensor.matmul(
                                    pL[:], hk[k][:, bi], dk[k][:],
                                    start=first, stop=False,
                                )
                                first = False
                        nc.tensor.matmul(
                            pL[:], w4_s[:, bi], dn4_s[:, sl_n], start=first, stop=True
                        )

                        if "stats" in skip:
                            continue
                        # online softmax stats
                        mt = tp.tile([128, 1], F32, tag="mt")
                        nc.vector.tensor_reduce(mt[:], pL[:], axis=AX.X, op=OP.max)
                        dlt = tp.tile([128, 1], F32, tag="dlt")
                        # dlt = min(m_old - mt, 0) = m_old - m_new
                        nc.vector.tensor_scalar(
                            out=dlt[:], in0=m_run[:, i:i + 1], scalar1=mt[:],
                            scalar2=0.0, op0=OP.subtract, op1=OP.min,
                        )
                        nc.vector.tensor_tensor(
                            m_run[:, i:i + 1], m_run[:, i:i + 1], mt[:], op=OP.max
                        )
                        alpha = tp.tile([128, 1], F32, tag="alpha")
                        nc.scalar.activation(
                            alpha[:], dlt[:], AF.Exp, bias=0.0, scale=ucol[:, i:i + 1]
                        )
                        ebias = tp.tile([128, 1], F32, tag="ebias")
                        nc.vector.tensor_tensor(
                            ebias[:], nucol[:, i:i + 1], m_run[:, i:i + 1], op=OP.mult
                        )
                        # prob = exp(u*P + bias) in bf16, lt = rowsum
                        prob = sfp.tile([128, NTILE], BF16, tag="prob")
                        lt = tp.tile([128, 1], F32, tag="lt")
                        nc.scalar.activation(
                            prob[:], pL[:], AF.Exp,
                            bias=ebias[:], scale=ucol[:, i:i + 1], accum_out=lt[:],
                        )
                        # l = l*alpha + lt (fused DVE)
                        nc.vector.scalar_tensor_tensor(
                            out=l_run[:, i:i + 1], in0=l_run[:, i:i + 1],
                            scalar=alpha[:], in1=lt[:], op0=OP.mult, op1=OP.add,
                        )
                        if "tail" in skip:
                            continue
                        if pending is not None:
                            emit_tail(*pending)
                        pending = (i, prob, alpha, nb)

                if pending is not None:
                    emit_tail(*pending)

                # ---------------- write partial state ----------------
                nc.sync.dma_start(out=m_out.ap(), in_=m_run[:])
                nc.sync.dma_start(out=l_out.ap(), in_=l_run[:])
                for i in range(NBT):
                    nc.sync.dma_start(out=acc_out_t[i], in_=acc[i][:])

    nc.compile()
    return nc


_BUILD_CACHE = {}


def _get_nc(n_tiles, tiers, reps=1, ndev=NCORES, skip=()):
    key = (n_tiles, tuple(tiers), reps, ndev, tuple(skip))
    if key not in _BUILD_CACHE:
        _BUILD_CACHE[key] = build(n_tiles, tuple(tiers), reps=reps, ndev=ndev, skip=tuple(skip))
    return _BUILD_CACHE[key]


def _rne11(x):
    """Round fp32 to 11 explicit mantissa bits (f32r-exact), round-half-even."""
    xi = np.ascontiguousarray(x, dtype=np.float32).view(np.uint32)
    keep = np.uint32(0xFFFFF000)
    half = np.uint32(0x800)
    odd = (xi >> np.uint32(12)) & np.uint32(1)
    r = (xi + (half - np.uint32(1)) + odd) & keep
    return r.view(np.float32)


def _trunc11(x):
    xi = np.ascontiguousarray(x, dtype=np.float32).view(np.uint32)
    return (xi & np.uint32(0xFFFFF000)).view(np.float32)


def _split11(x):
    hi = _rne11(x)
    lo = _trunc11((x.astype(np.float32) - hi))
    return hi, lo


def prepare(x_t, t, dataset, n_tiles):
    """Host-side layout: sort rows by t, pad+shard dataset, pre-split."""
    bf16 = ml_dtypes.bfloat16
    n = dataset.shape[0]
    n_pad = NCORES * n_tiles * NTILE
    assert n_pad >= n

    perm = np.argsort(t, kind="stable")
    xs = np.ascontiguousarray(x_t[perm])
    ts = t[perm].astype(np.float64)

    a = ts
    b = 1.0 - a
    u = (a / (b * b)).astype(np.float32)
    w = (-a / 2.0).astype(np.float32)
    dcoef = (1.0 + a / b)
    vcoef = (-1.0 / b)

    # per-b-tile pass tiers from max u in tile
    umax = u.reshape(NBT, 128).max(axis=1)
    tiers = tuple(1 if um <= U_1PASS else 3 for um in umax)

    # dataset: pad with far-away rows (value 0, huge norm)
    dn = np.einsum("nd,nd->n", dataset, dataset, dtype=np.float64).astype(np.float32)
    dn_pad = np.full(n_pad, 1.0e6, dtype=np.float32)
    dn_pad[:n] = dn
    dpad = np.zeros((n_pad, D), dtype=np.float32)
    dpad[:n] = dataset

    ds_hi32, ds_lo32 = _split11(dpad)
    # transposed splits: [core, t, k, p(128 d), NTILE]
    def tsplit(z):
        # z: [n_pad, D] f32 -> [core, t, k, 128, NTILE]
        zt = z.T.reshape(4, 128, NCORES, n_tiles, NTILE)
        return np.ascontiguousarray(zt.transpose(2, 3, 0, 1, 4))

    dsth = tsplit(ds_hi32)
    dstl = tsplit(ds_lo32)
    natb = np.ascontiguousarray(dpad.astype(bf16)).reshape(
        NCORES, n_tiles * 4 * 128, D
    )

    dnh, dnl = _split11(dn_pad)
    dn4 = np.stack([dnh, dnl, dnh, dnl]).reshape(4, NCORES, n_tiles * NTILE)
    wh, wl = _split11(w)
    w4 = np.ascontiguousarray(np.stack([wh, wh, wl, wl]))

    xT = xs.T.reshape(4, 128, B)
    xh, xl = _split11(xT)

    def col(v):
        return np.ascontiguousarray(v.astype(np.float32).reshape(NBT, 128).T)

    base = dict(
        xh=xh.reshape(4 * 128, B),
        xl=xl.reshape(4 * 128, B),
        w4=w4,
        ucol=col(u),
        nucol=col(-u),
    )
    in_maps = [
        dict(
            base,
            dsth=dsth[c].reshape(n_tiles * 4 * 128, NTILE),
            dstl=dstl[c].reshape(n_tiles * 4 * 128, NTILE),
            natb=natb[c],
            dn4=np.ascontiguousarray(dn4[:, c]),
        )
        for c in range(NCORES)
    ]
    aux = dict(perm=perm, u=u, dcoef=dcoef, vcoef=vcoef, xs=xs, tiers=tiers)
    return in_maps, aux


def merge(results, aux):
    """Host-side flash-softmax merge of per-core partials -> full output."""
    u = aux["u"].astype(np.float64)  # [B] sorted order
    # device layout: [128, NBT] col i holds b = i*128+p -> transpose+flatten
    def uncol(z):
        return np.asarray(z, dtype=np.float64).T.reshape(B)

    ms = np.stack([uncol(r["m_out"]) for r in results])  # [C, B]
    ls = np.stack([uncol(r["l_out"]) for r in results])  # [C, B]
    accs = np.stack([np.asarray(r["acc_out"], dtype=np.float64) for r in results])

    m_glob = ms.max(axis=0)  # [B]
    gam = np.exp(u[None, :] * (ms - m_glob[None, :]))  # [C, B]
    l_glob = (gam * ls).sum(axis=0)  # [B]
    acc_glob = np.einsum("cb,cbd->bd", gam, accs)  # [B, D]

    v_sorted = (
        aux["dcoef"][:, None] * acc_glob / l_glob[:, None]
        + aux["vcoef"][:, None] * aux["xs"].astype(np.float64)
    )
    v = np.empty((B, D), dtype=np.float32)
    v[aux["perm"]] = v_sorted.astype(np.float32)
    return v


def kernel(x_t, t, dataset):
    x_t = np.asarray(x_t, dtype=np.float32)
    t = np.asarray(t, dtype=np.float32)
    dataset = np.asarray(dataset, dtype=np.float32)
    n = dataset.shape[0]
    n_tiles = -(-n // (NCORES * NTILE))  # ceil -> 25 for N=100000
    in_maps, aux = prepare(x_t, t, dataset, n_tiles)
    nc = _get_nc(n_tiles, aux["tiers"])
    res = run_bass_kernel_spmd(nc, in_maps, core_ids=list(range(NCORES)))
    return merge(res.results, aux)


def ref_numpy(x_t, t, dataset):
    aa = t.astype(np.float64)
    bb = 1.0 - aa
    dsn = (dataset.astype(np.float64) ** 2).sum(1)
    t2 = x_t.astype(np.float64) @ dataset.T.astype(np.float64)
    logit = (-1.0 / (2 * bb * bb))[:, None] * (
        (x_t.astype(np.float64) ** 2).sum(1)[:, None]
        - 2 * aa[:, None] * t2
        + (aa * aa)[:, None] * dsn[None, :]
    )
    p = np.exp(logit - logit.max(1, keepdims=True))
    p /= p.sum(1, keepdims=True)
    p = p * (1 + aa / bb)[:, None]
    return (-1.0 / bb)[:, None] * x_t.astype(np.float64) + p @ dataset.astype(np.float64)


if __name__ == "__main__":
    rng = np.random.default_rng(0)
    n = 2 * NCORES * NTILE - 300
    x_t = rng.standard_normal((B, D)).astype(np.float32)
    t = rng.uniform(0.05, 0.95, (B,)).astype(np.float32)
    dataset = rng.standard_normal((n, D)).astype(np.float32)
    v = kernel(x_t, t, dataset)
    vref = ref_numpy(x_t, t, dataset)
    err = np.linalg.norm(v - vref) / np.linalg.norm(vref)
    print("rel l2 err:", err)
    print("max abs err:", np.abs(v - vref).max(), "ref scale:", np.abs(vref).max())


# revision 10
# speedup vs baseline: 1020.6186x; 1.0311x over previous
"""AnalyticGaussianVelocity (soft-kNN flow velocity) on 8 trn2 NeuronCores.

Math (reference):
    a = t, b = 1-t
    logit[b,n] = -1/(2 b^2) * ||x_b - a * d_n||^2
    prob = softmax(logit, axis=n) * (1 + a/b)
    v = (-1/b) x + prob @ dataset

Dropping per-row constants, softmax(logit) == softmax(u * P) with
    u = a/b^2  (>0),  P[b,n] = x_b . d_n - (a/2) ||d_n||^2

Kernel strategy (v2):
  - dataset sharded over N across 8 cores; per-core flash-style online
    softmax; partial (m, l, acc) returned per core and merged on HOST
    (no collectives on device).
  - all layout work on host: dataset pre-transposed and pre-split into
    f32r-exact 11-bit hi/lo components (f32r matmul is exact for 11-bit
    mantissas and runs 1 cyc/row like bf16); norms and -(a/2) rows
    pre-split 2-way 11-bit (K=4 exact norm matmul).
  - rows sorted by t on host so that precision tiers align with b-tiles:
    logit abs error tolerance is ~0.1/u with u = a/(1-a)^2, so low-u
    b-tiles need only the single hi*hi matmul pass (12-bit operands,
    eps ~ 3e-3), while high-u b-tiles use 3 passes (hi*hi + hi*lo +
    lo*hi, eps ~ 6e-7). Pass count chosen per b-tile from its max u.
  - MM2: prob (bf16 from ACT exp) -> PE bf16 transposes -> probT @
    dataset_bf16; accumulator update acc = alpha*acc + pA fused on DVE
    reading PSUM directly (no diag-rescale matmul, no ACT acc copy).
"""

import sys

sys.path.insert(0, "/opt/trn_rl_repo")

import numpy as np
import ml_dtypes

import concourse.bass as bass
import concourse.mybir as mybir
import concourse.tile as tile
from concourse import bacc
from concourse.bass_utils import run_bass_kernel_spmd
from concourse.masks import make_identity

B, D = 1024, 512
NCORES = 8
NTILE = 512  # dataset rows per n-tile
NBT = B // 128  # 8 b-tiles

F32 = mybir.dt.float32
F32R = mybir.dt.float32r
BF16 = mybir.dt.bfloat16

AF = mybir.ActivationFunctionType
OP = mybir.AluOpType
AX = mybir.AxisListType

# mode thresholds on per-b-tile max u (logit abs-error budget ~0.1):
# "b": single bf16xbf16 pass (noise ~0.036)   for u <= U_BF16
# "f": single f32r hi*hi pass (noise ~3e-3)   for u <= U_1PASS
# "3": f32r hi*hi + bf16 correction passes (noise ~2e-4) otherwise
U_BF16 = 3.0
U_1PASS = 33.0


def build(n_tiles, tiers, reps=1, ndev=NCORES, skip=()):
    """tiers: NBT ints in {1,3} — MM1 passes per sorted b-tile.
    skip: subset of {"stats","tail","mm1"} for timing-attribution builds."""
    n_sh = n_tiles * NTILE
    nc = bacc.Bacc("TRN2", target_bir_lowering=False, debug=False, num_devices=ndev)

    # --- dram params (per core) ---
    # transposed dataset hi/lo, layout [t, k(128-chunk of d), 128 d, NTILE n]
    dsth_p = nc.declare_dram_parameter("dsth", [n_tiles * 4 * 128, NTILE], F32R, isOutput=False)
    dsthb_p = nc.declare_dram_parameter("dsthb", [n_tiles * 4 * 128, NTILE], BF16, isOutput=False)
    dstlb_p = nc.declare_dram_parameter("dstlb", [n_tiles * 4 * 128, NTILE], BF16, isOutput=False)
    # row-major bf16 dataset for MM2, layout [t, j(128-chunk of n), 128 n, D]
    natb_p = nc.declare_dram_parameter("natb", [n_tiles * 4 * 128, D], BF16, isOutput=False)
    # norm rows (wh.dnh + wh.dnl + wl.dnh + wl.dnl): dn4 = (dnh,dnl,dnh,dnl)
    dn4_p = nc.declare_dram_parameter("dn4", [4, n_sh], F32R, isOutput=False)
    # w4 = (wh,wh,wl,wl) where w = -(a/2) per sorted b row, [4, B]
    w4_p = nc.declare_dram_parameter("w4", [4, B], F32R, isOutput=False)
    # x^T hi/lo splits, layout [k(128-chunk of d), 128 d, B]
    xh_p = nc.declare_dram_parameter("xh", [4 * 128, B], F32R, isOutput=False)
    xb_p = nc.declare_dram_parameter("xb", [4 * 128, B], BF16, isOutput=False)
    xlb_p = nc.declare_dram_parameter("xlb", [4 * 128, B], BF16, isOutput=False)
    # per-b coefficient columns [128, NBT]: col i holds b = i*128+p (sorted order)
    ucol_p = nc.declare_dram_parameter("ucol", [128, NBT], F32, isOutput=False)
    nucol_p = nc.declare_dram_parameter("nucol", [128, NBT], F32, isOutput=False)
    # outputs: per-core partial softmax state
    m_out = nc.declare_dram_parameter("m_out", [128, NBT], F32, isOutput=True)
    l_out = nc.declare_dram_parameter("l_out", [128, NBT], F32, isOutput=True)
    acc_out = nc.declare_dram_parameter("acc_out", [NBT * 128, D], F32, isOutput=True)

    dsth_t = dsth_p.ap().rearrange("(t k p) n -> t k p n", k=4, p=128)
    dsthb_t = dsthb_p.ap().rearrange("(t k p) n -> t k p n", k=4, p=128)
    dstlb_t = dstlb_p.ap().rearrange("(t k p) n -> t k p n", k=4, p=128)
    natb_t = natb_p.ap().rearrange("(t j p) d -> t j p d", j=4, p=128)
    xh_t = xh_p.ap().rearrange("(k p) b -> k p b", p=128)
    xb_t = xb_p.ap().rearrange("(k p) b -> k p b", p=128)
    xlb_t = xlb_p.ap().rearrange("(k p) b -> k p b", p=128)
    acc_out_t = acc_out.ap().rearrange("(i p) d -> i p d", p=128)

    with tile.TileContext(nc) as tc:
        with (
            tc.tile_pool(name="persist", bufs=1) as pp,
            tc.tile_pool(name="dt", bufs=2) as dtp,
            tc.tile_pool(name="nat", bufs=2) as natp,
            tc.tile_pool(name="sf", bufs=4) as sfp,
            tc.tile_pool(name="tiny", bufs=6) as tp,
            tc.tile_pool(name="psL", bufs=3, space="PSUM") as psL,
            tc.tile_pool(name="psA", bufs=2, space="PSUM") as psA,
            tc.tile_pool(name="psT", bufs=2, space="PSUM") as psT,
        ):
            # ---------------- resident setup ----------------
            ident = pp.tile([128, 128], F32)
            make_identity(nc, ident[:])
            ident_bf = pp.tile([128, 128], BF16)
            nc.vector.tensor_copy(ident_bf[:], ident[:])

            xh_s = [pp.tile([128, B], F32R, name=f"xh{k}") for k in range(4)]
            xb_s = [pp.tile([128, B], BF16, name=f"xb{k}") for k in range(4)]
            xlb_s = [pp.tile([128, B], BF16, name=f"xlb{k}") for k in range(4)]
            for k in range(4):
                nc.sync.dma_start(out=xh_s[k][:], in_=xh_t[k])
                nc.sync.dma_start(out=xb_s[k][:], in_=xb_t[k])
            for k in range(4):
                nc.sync.dma_start(out=xlb_s[k][:], in_=xlb_t[k])
            w4_s = pp.tile([4, B], F32R)
            nc.sync.dma_start(out=w4_s[:], in_=w4_p.ap())
            dn4_s = pp.tile([4, n_sh], F32R)
            nc.sync.dma_start(out=dn4_s[:], in_=dn4_p.ap())
            ucol = pp.tile([128, NBT], F32)
            nucol = pp.tile([128, NBT], F32)
            nc.sync.dma_start(out=ucol[:], in_=ucol_p.ap())
            nc.sync.dma_start(out=nucol[:], in_=nucol_p.ap())

            m_run = pp.tile([128, NBT], F32)
            l_run = pp.tile([128, NBT], F32)
            acc = [pp.tile([128, D], F32, name=f"acc{i}") for i in range(NBT)]

            for _rep in range(reps):
                nc.vector.memset(m_run[:], -1.0e30)
                nc.vector.memset(l_run[:], 0.0)
                for i in range(NBT):
                    nc.vector.memset(acc[i][:], 0.0)
                pending = None

                # ---------------- main loop over dataset tiles ----------------
                for t in range(n_tiles):
                    dTh = [dtp.tile([128, NTILE], F32R, tag=f"dTh{k}", name=f"dTh{k}") for k in range(4)]
                    dThb = [dtp.tile([128, NTILE], BF16, tag=f"dThb{k}", name=f"dThb{k}") for k in range(4)]
                    dTlb = [dtp.tile([128, NTILE], BF16, tag=f"dTlb{k}", name=f"dTlb{k}") for k in range(4)]
                    nb = [natp.tile([128, D], BF16, tag=f"nb{j}", name=f"nb{j}") for j in range(4)]
                    for k in range(4):
                        nc.sync.dma_start(out=dTh[k][:], in_=dsth_t[t, k])
                        nc.sync.dma_start(out=dThb[k][:], in_=dsthb_t[t, k])
                        nc.sync.dma_start(out=dTlb[k][:], in_=dstlb_t[t, k])
                        nc.sync.dma_start(out=nb[k][:], in_=natb_t[t, k])
                    sl_n = slice(t * NTILE, (t + 1) * NTILE)

                    def emit_tail(i, prob, alpha, nb):
                        """PE tail of b-tile i: probT transposes + MM2, then
                        the DVE accumulator update. Emitted one b-tile late
                        so PE never waits on the softmax chain."""
                        pP = psT.tile([128, NTILE], BF16, tag="pP", name="pP")
                        for k in range(4):
                            ksl = slice(k * 128, (k + 1) * 128)
                            nc.tensor.transpose(pP[:, ksl], prob[:, ksl], ident_bf[:])
                        probT = sfp.tile([128, NTILE], BF16, tag="probT", name="probT")
                        nc.scalar.copy(probT[:], pP[:])
                        pA = psA.tile([128, D], F32, tag="pA", name="pA")
                        for k in range(4):
                            ksl = slice(k * 128, (k + 1) * 128)
                            nc.tensor.matmul(
                                pA[:], probT[:, ksl], nb[k][:],
                                start=(k == 0), stop=(k == 3),
                            )
                        # acc = alpha*acc + pA (fused DVE, reads PSUM)
                        nc.vector.scalar_tensor_tensor(
                            out=acc[i][:], in0=acc[i][:],
                            scalar=alpha[:], in1=pA[:], op0=OP.mult, op1=OP.add,
                        )

                    for i in range(NBT):
                        bi = slice(i * 128, (i + 1) * 128)
                        pL = psL.tile([128, NTILE], F32, tag="pL")
                        mode = tiers[i]
                        if mode == "b":
                            passes = ((xb_s, dThb),)
                        elif mode == "f":
                            passes = ((xh_s, dTh),)
                        else:
                            passes = ((xh_s, dTh), (xb_s, dTlb), (xlb_s, dThb))
                        if "mm1" in skip:
                            passes = ()
                        first = True
                        for hk, dk in passes:
                            for k in range(4):
                                nc.tensor.matmul(
                                    pL[:], hk[k][:, bi], dk[k][:],
                                    start=first, stop=False,
                                )
                                first = False
                        nc.tensor.matmul(
                            pL[:], w4_s[:, bi], dn4_s[:, sl_n], start=first, stop=True
                        )

                        if "stats" in skip:
                            continue
                        # online softmax stats
                        mt = tp.tile([128, 1], F32, tag="mt")
                        nc.vector.tensor_reduce(mt[:], pL[:], axis=AX.X, op=OP.max)
                        dlt = tp.tile([128, 1], F32, tag="dlt")
                        # dlt = min(m_old - mt, 0) = m_old - m_new
                        nc.vector.tensor_scalar(
                            out=dlt[:], in0=m_run[:, i:i + 1], scalar1=mt[:],
                            scalar2=0.0, op0=OP.subtract, op1=OP.min,
                        )
                        nc.vector.tensor_tensor(
                            m_run[:, i:i + 1], m_run[:, i:i + 1], mt[:], op=OP.max
                        )
                        alpha = tp.tile([128, 1], F32, tag="alpha")
                        nc.scalar.activation(
                            alpha[:], dlt[:], AF.Exp, bias=0.0, scale=ucol[:, i:i + 1]
                        )
                        ebias = tp.tile([128, 1], F32, tag="ebias")
                        nc.vector.tensor_tensor(
                            ebias[:], nucol[:, i:i + 1], m_run[:, i:i + 1], op=OP.mult
                        )
                        # prob = exp(u*P + bias) in bf16, lt = rowsum
                        prob = sfp.tile([128, NTILE], BF16, tag="prob")
                        lt = tp.tile([128, 1], F32, tag="lt")
                        nc.scalar.activation(
                            prob[:], pL[:], AF.Exp,
                            bias=ebias[:], scale=ucol[:, i:i + 1], accum_out=lt[:],
                        )
                        # l = l*alpha + lt (fused DVE)
                        nc.vector.scalar_tensor_tensor(
                            out=l_run[:, i:i + 1], in0=l_run[:, i:i + 1],
                            scalar=alpha[:], in1=lt[:], op0=OP.mult, op1=OP.add,
                        )
                        if "tail" in skip:
                            continue
                        if pending is not None:
                            emit_tail(*pending)
                        pending = (i, prob, alpha, nb)

                if pending is not None:
                    emit_tail(*pending)

                # ---------------- write partial state ----------------
                nc.sync.dma_start(out=m_out.ap(), in_=m_run[:])
                nc.sync.dma_start(out=l_out.ap(), in_=l_run[:])
                for i in range(NBT):
                    nc.sync.dma_start(out=acc_out_t[i], in_=acc[i][:])

    nc.compile()
    return nc


_BUILD_CACHE = {}


def _get_nc(n_tiles, tiers, reps=1, ndev=NCORES, skip=()):
    key = (n_tiles, tuple(tiers), reps, ndev, tuple(skip))
    if key not in _BUILD_CACHE:
        _BUILD_CACHE[key] = build(n_tiles, tuple(tiers), reps=reps, ndev=ndev, skip=tuple(skip))
    return _BUILD_CACHE[key]


def _rne11(x):
    """Round fp32 to 11 explicit mantissa bits (f32r-exact), round-half-even."""
    xi = np.ascontiguousarray(x, dtype=np.float32).view(np.uint32)
    keep = np.uint32(0xFFFFF000)
    half = np.uint32(0x800)
    odd = (xi >> np.uint32(12)) & np.uint32(1)
    r = (xi + (half - np.uint32(1)) + odd) & keep
    return r.view(np.float32)


def _trunc11(x):
    xi = np.ascontiguousarray(x, dtype=np.float32).view(np.uint32)
    return (xi & np.uint32(0xFFFFF000)).view(np.float32)


def _split11(x):
    hi = _rne11(x)
    lo = _trunc11((x.astype(np.float32) - hi))
    return hi, lo


def prepare(x_t, t, dataset, n_tiles):
    """Host-side layout: sort rows by t, pad+shard dataset, pre-split."""
    bf16 = ml_dtypes.bfloat16
    n = dataset.shape[0]
    n_pad = NCORES * n_tiles * NTILE
    assert n_pad >= n

    perm = np.argsort(t, kind="stable")
    xs = np.ascontiguousarray(x_t[perm])
    ts = t[perm].astype(np.float64)

    a = ts
    b = 1.0 - a
    u = (a / (b * b)).astype(np.float32)
    w = (-a / 2.0).astype(np.float32)
    dcoef = (1.0 + a / b)
    vcoef = (-1.0 / b)

    # per-b-tile MM1 mode from max u in tile
    umax = u.reshape(NBT, 128).max(axis=1)
    tiers = tuple(
        "b" if um <= U_BF16 else ("f" if um <= U_1PASS else "3") for um in umax
    )

    # dataset: pad with far-away rows (value 0, huge norm)
    dn = np.einsum("nd,nd->n", dataset, dataset, dtype=np.float64).astype(np.float32)
    dn_pad = np.full(n_pad, 1.0e6, dtype=np.float32)
    dn_pad[:n] = dn
    dpad = np.zeros((n_pad, D), dtype=np.float32)
    dpad[:n] = dataset

    ds_hi32, ds_lo32 = _split11(dpad)
    # transposed layouts: [core, t, k, p(128 d), NTILE]
    def tsplit(z):
        # z: [n_pad, D] -> [core, t, k, 128, NTILE]
        zt = z.T.reshape(4, 128, NCORES, n_tiles, NTILE)
        return np.ascontiguousarray(zt.transpose(2, 3, 0, 1, 4))

    dsth = tsplit(ds_hi32)
    dsthb = tsplit(dpad.astype(bf16))
    dstlb = tsplit(ds_lo32.astype(bf16))
    natb = np.ascontiguousarray(dpad.astype(bf16)).reshape(
        NCORES, n_tiles * 4 * 128, D
    )

    dnh, dnl = _split11(dn_pad)
    dn4 = np.stack([dnh, dnl, dnh, dnl]).reshape(4, NCORES, n_tiles * NTILE)
    wh, wl = _split11(w)
    w4 = np.ascontiguousarray(np.stack([wh, wh, wl, wl]))

    xT = np.ascontiguousarray(xs.T).reshape(4, 128, B)
    xh, xl = _split11(xT)

    def col(v):
        return np.ascontiguousarray(v.astype(np.float32).reshape(NBT, 128).T)

    base = dict(
        xh=xh.reshape(4 * 128, B),
        xb=np.ascontiguousarray(xT.astype(bf16)).reshape(4 * 128, B),
        xlb=xl.astype(bf16).reshape(4 * 128, B),
        w4=w4,
        ucol=col(u),
        nucol=col(-u),
    )
    in_maps = [
        dict(
            base,
            dsth=dsth[c].reshape(n_tiles * 4 * 128, NTILE),
            dsthb=dsthb[c].reshape(n_tiles * 4 * 128, NTILE),
            dstlb=dstlb[c].reshape(n_tiles * 4 * 128, NTILE),
            natb=natb[c],
            dn4=np.ascontiguousarray(dn4[:, c]),
        )
        for c in range(NCORES)
    ]
    aux = dict(perm=perm, u=u, dcoef=dcoef, vcoef=vcoef, xs=xs, tiers=tiers)
    return in_maps, aux


def merge(results, aux):
    """Host-side flash-softmax merge of per-core partials -> full output."""
    u = aux["u"].astype(np.float64)  # [B] sorted order
    # device layout: [128, NBT] col i holds b = i*128+p -> transpose+flatten
    def uncol(z):
        return np.asarray(z, dtype=np.float64).T.reshape(B)

    ms = np.stack([uncol(r["m_out"]) for r in results])  # [C, B]
    ls = np.stack([uncol(r["l_out"]) for r in results])  # [C, B]
    accs = np.stack([np.asarray(r["acc_out"], dtype=np.float64) for r in results])

    m_glob = ms.max(axis=0)  # [B]
    gam = np.exp(u[None, :] * (ms - m_glob[None, :]))  # [C, B]
    l_glob = (gam * ls).sum(axis=0)  # [B]
    acc_glob = np.einsum("cb,cbd->bd", gam, accs)  # [B, D]

    v_sorted = (
        aux["dcoef"][:, None] * acc_glob / l_glob[:, None]
        + aux["vcoef"][:, None] * aux["xs"].astype(np.float64)
    )
    v = np.empty((B, D), dtype=np.float32)
    v[aux["perm"]] = v_sorted.astype(np.float32)
    return v


def kernel(x_t, t, dataset):
    x_t = np.asarray(x_t, dtype=np.float32)
    t = np.asarray(t, dtype=np.float32)
    dataset = np.asarray(dataset, dtype=np.float32)
    n = dataset.shape[0]
    n_tiles = -(-n // (NCORES * NTILE))  # ceil -> 25 for N=100000
    in_maps, aux = prepare(x_t, t, dataset, n_tiles)
    nc = _get_nc(n_tiles, aux["tiers"])
    res = run_bass_kernel_spmd(nc, in_maps, core_ids=list(range(NCORES)))
    return merge(res.results, aux)


def ref_numpy(x_t, t, dataset):
    aa = t.astype(np.float64)
    bb = 1.0 - aa
    dsn = (dataset.astype(np.float64) ** 2).sum(1)
    t2 = x_t.astype(np.float64) @ dataset.T.astype(np.float64)
    logit = (-1.0 / (2 * bb * bb))[:, None] * (
        (x_t.astype(np.float64) ** 2).sum(1)[:, None]
        - 2 * aa[:, None] * t2
        + (aa * aa)[:, None] * dsn[None, :]
    )
    p = np.exp(logit - logit.max(1, keepdims=True))
    p /= p.sum(1, keepdims=True)
    p = p * (1 + aa / bb)[:, None]
    return (-1.0 / bb)[:, None] * x_t.astype(np.float64) + p @ dataset.astype(np.float64)


if __name__ == "__main__":
    rng = np.random.default_rng(0)
    n = 2 * NCORES * NTILE - 300
    x_t = rng.standard_normal((B, D)).astype(np.float32)
    t = rng.uniform(0.05, 0.95, (B,)).astype(np.float32)
    dataset = rng.standard_normal((n, D)).astype(np.float32)
    v = kernel(x_t, t, dataset)
    vref = ref_numpy(x_t, t, dataset)
    err = np.linalg.norm(v - vref) / np.linalg.norm(vref)
    print("rel l2 err:", err)
    print("max abs err:", np.abs(v - vref).max(), "ref scale:", np.abs(vref).max())
